# revision 40
# baseline (speedup 1.0000x reference)
"""AsymmetricSVD segment-reduce kernel for 8 TRN2 NeuronCores.

Strategy (data-parallel over segments):
  - Core m owns segments [512m, 512(m+1)) and their contiguous implicit
    entries (segment_ids is sorted).
  - Host precomputes per-entry scalar a_e = r_e - MU - bu[user[seg_e]] and a
    fused bf16 table XY = [X | Y - bi*X] (so w*X + Y == a*X + Y').
  - Device gathers 512B rows of XY per entry via gpsimd.dma_gather
    (hardware SWDGE gather; int16 indices -> 4 item-range buckets of 25000
    rows each) and builds near-one-hot segment matrices from iota-vs-segoff
    compares.
  - Feature-major accumulation: PE matmuls put features in PSUM partitions
    and segments along the PSUM free dim (arbitrary offsets allowed there):
        psumX[f, seg] += sum_e a_e * X_e[f]     (lhsT = X rows, rhs = S*a)
        psumY[f, seg] += sum_e Y'_e[f]          (lhsT = Y' rows, rhs = S)
  - Epilogue: rui[seg] = bui[seg] + sum_f QnT[f, seg]*(psumX+psumY)[f, seg]
    via one elementwise multiply and a ones-vector matmul over partitions.

The Bass graph is traced per call (uniform across the 8 cores; only tensor
values differ per core), compiled, and run via run_bass_kernel_spmd.
"""

import numpy as np
import ml_dtypes

MU = 3.5
B = 4096
F = 128
NUM_ITEMS = 100000
N_CORES = 8
SEGS_PER_CORE = B // N_CORES            # 512
N_BUCKETS = 4
BUCKET_ROWS = (NUM_ITEMS + N_BUCKETS - 1) // N_BUCKETS   # 25000 < 32768 (int16)
CHUNK = 3584                             # entries per dma_gather call
PAD_SEG = -(10 ** 6)                     # int sentinel for padding entries
XY_SCALE = 64.0                          # fp8 range scaling (undone via qnT)

def _host_prep(bu, bi, Q, X, Y, user, item, imp_items, imp_ratings, segment_ids):
    """All index/scalar preprocessing. Returns per-core device arrays and
    group metadata for codegen."""
    T = imp_items.shape[0]

    # per-entry scalar (weight minus the per-item part, which is folded into Y')
    a_full = imp_ratings.astype(np.float32) - MU - bu[user[segment_ids], 0]
    Yp = Y - bi * X                                   # [NUM_ITEMS, F]
    XY = np.clip(
        np.concatenate([X, Yp], axis=1) * XY_SCALE, -240.0, 240.0
    ).astype(ml_dtypes.float8_e4m3)                   # [NI, 256]

    counts = np.bincount(segment_ids, minlength=B).astype(np.float32)
    norm = np.where(counts > 0, counts, 1.0) ** -0.5
    bui = (MU + bu[user, 0] + bi[item, 0]).astype(np.float32)        # [B]
    Qn = (Q[item] * norm[:, None]).astype(np.float32)                # [B, F]

    # --- shard entries by segment block; bucket-stable-sort by item range ---
    bounds = np.searchsorted(segment_ids, np.arange(0, B + 1, SEGS_PER_CORE))
    cores = []
    for m in range(N_CORES):
        lo, hi = bounds[m], bounds[m + 1]
        it = imp_items[lo:hi]
        sl = (segment_ids[lo:hi] - m * SEGS_PER_CORE).astype(np.int64)
        av = a_full[lo:hi]
        bk = it // BUCKET_ROWS
        order = np.argsort(bk, kind="stable")
        it, sl, av, bk = it[order], sl[order], av[order], bk[order]
        bcnt = np.bincount(bk, minlength=N_BUCKETS)
        cores.append((it, sl, av, bcnt))

    cap = np.zeros(N_BUCKETS, np.int64)
    for m in range(N_CORES):
        cap = np.maximum(cap, cores[m][3])
    cap = ((cap + 127) // 128) * 128                    # per-bucket capacity
    offs = np.concatenate([[0], np.cumsum(cap)])
    E_pad = int(offs[-1])
    G = E_pad // 128

    # padded per-core streams
    lidx = np.zeros((N_CORES, E_pad), np.int16)          # local row in bucket
    segl = np.full((N_CORES, E_pad), PAD_SEG, np.int64)  # local segment id
    aval = np.zeros((N_CORES, E_pad), np.float32)
    for m in range(N_CORES):
        it, sl, av, bcnt = cores[m]
        pos = 0
        for b in range(N_BUCKETS):
            n = int(bcnt[b])
            d = int(offs[b])
            lidx[m, d:d + n] = (it[pos:pos + n] - b * BUCKET_ROWS).astype(np.int16)
            segl[m, d:d + n] = sl[pos:pos + n]
            aval[m, d:d + n] = av[pos:pos + n]
            pos += n

    # --- group metadata (cross-core, uniform) ---
    sg = segl.reshape(N_CORES, G, 128)
    real = sg != PAD_SEG
    any_real = real.any(axis=(0, 2))                     # [G]
    lo_g = np.where(real, sg, 10 ** 9).min(axis=(0, 2))
    hi_g = np.where(real, sg, -1).max(axis=(0, 2))
    A_g = np.where(any_real, np.minimum(lo_g, 10 ** 9 - 1), 0)
    offmax = np.where(any_real, hi_g - A_g, 0)
    mwin = int(max(16, ((offmax.max() + 8) // 8) * 8)) if any_real.any() else 16
    if mwin > 128:
        raise RuntimeError(f"pathological segment distribution: mwin={mwin}")

    # device segoff values
    segoff = np.where(
        real, sg - A_g[None, :, None], -1000
    ).astype(ml_dtypes.bfloat16)                          # [N_CORES, G, 128]

    # chunk list: (start_entry, n_entries, bucket) — near-equal chunks per
    # bucket (multiples of 128) to avoid undersized tail calls
    chunks = []
    for b in range(N_BUCKETS):
        s, e = int(offs[b]), int(offs[b + 1])
        total_g = (e - s) // 128
        if total_g == 0:
            continue
        ncalls = max(1, (total_g * 128 + CHUNK - 1) // CHUNK)
        base_g, extra = divmod(total_g, ncalls)
        for c in range(ncalls):
            g = base_g + (1 if c < extra else 0)
            n = g * 128
            chunks.append((s, n, b))
            s += n

    # halve the last 8 chunks so the trailing drains (which run after the
    # final descriptor-gen with nothing left to overlap) are short
    if len(chunks) >= 8:
        tail, chunks = chunks[-8:], chunks[:-8]
        for (s, n, b) in tail:
            h = (n // 256) * 128
            if h and n - h:
                chunks.append((s, h, b))
                chunks.append((s + h, n - h, b))
            else:
                chunks.append((s, n, b))

    meta = dict(
        E_pad=E_pad, G=G, mwin=mwin, chunks=chunks,
        A=A_g.astype(np.int64), any_real=any_real,
        offmax=offmax.astype(np.int64),
    )

    # --- device arrays per core ---
    def wrap16(x):   # entry e -> [e%16, e//16], replicated to 128 partitions
        w = x.reshape(-1, 16).T
        return np.ascontiguousarray(np.tile(w, (8, 1)))

    def wrap128(x):  # entry e -> [e%128, e//128]
        return np.ascontiguousarray(x.reshape(-1, 128).T)

    iota = np.broadcast_to(
        np.arange(mwin, dtype=np.float32), (128, mwin)
    ).astype(ml_dtypes.bfloat16)

    in_maps = []
    for m in range(N_CORES):
        sl0 = m * SEGS_PER_CORE
        in_maps.append({
            "xy": XY,
            "iota": np.ascontiguousarray(iota),
            "widx": wrap16(np.arange(3584, dtype=np.int16)),
            "idx16": wrap16(lidx[m]),
            "segoff": wrap128_bf(segoff[m]),
            "aw": wrap128(aval[m]).astype(ml_dtypes.bfloat16),
            "qnT": np.ascontiguousarray(Qn[sl0:sl0 + SEGS_PER_CORE].T / XY_SCALE),
            "bui": np.ascontiguousarray(
                bui[sl0:sl0 + SEGS_PER_CORE].reshape(-1, 128).T),
        })
    return in_maps, meta


def wrap128_bf(x):
    return np.ascontiguousarray(np.asarray(x).reshape(-1, 128).T)


def _build_graph(meta, stage=4):
    # stage: 1=gathers only, 2=+S build, 3=+matmuls, 4=full epilogue
    from concourse import bacc, mybir
    from concourse.tile import TileContext

    E_pad, G, mwin = meta["E_pad"], meta["G"], meta["mwin"]
    chunks = meta["chunks"]
    A, any_real, offmax = meta["A"], meta["any_real"], meta["offmax"]

    nc = bacc.Bacc("TRN2", target_bir_lowering=False, debug=False,
                   num_devices=N_CORES, num_swdge_queues=4)
    bf16, f32, i16 = mybir.dt.bfloat16, mybir.dt.float32, mybir.dt.int16
    f8 = mybir.dt.float8e4

    xy_d = nc.declare_dram_parameter("xy", [NUM_ITEMS, 256], f8, isOutput=False)
    iota_d = nc.declare_dram_parameter("iota", [128, mwin], bf16, isOutput=False)
    idx_d = nc.declare_dram_parameter("idx16", [128, E_pad // 16], i16, isOutput=False)
    seg_d = nc.declare_dram_parameter("segoff", [128, G], bf16, isOutput=False)
    aw_d = nc.declare_dram_parameter("aw", [128, G], bf16, isOutput=False)
    qn_d = nc.declare_dram_parameter("qnT", [128, SEGS_PER_CORE], f32, isOutput=False)
    bui_d = nc.declare_dram_parameter("bui", [128, SEGS_PER_CORE // 128], f32,
                                      isOutput=False)
    widx_d = nc.declare_dram_parameter("widx", [128, 3584 // 16], i16,
                                       isOutput=False)
    out_d = nc.declare_dram_parameter("out", [SEGS_PER_CORE], f32, isOutput=True)

    with TileContext(nc) as tc:
        with (
            tc.tile_pool(name="const", bufs=1) as cpool,
            tc.tile_pool(name="xy", bufs=8) as xypool,
            tc.tile_pool(name="idx", bufs=6) as ipool,
            tc.tile_pool(name="sel", bufs=4) as spool,
            tc.tile_pool(name="epi", bufs=1) as epool,
            tc.tile_pool(name="psum", bufs=1, space="PSUM") as ppool,
        ):
            # non-critical preloads on the scalar engine's HWDGE queue so the
            # sync queue only carries the gather-gating idx slices
            iota_t = cpool.tile([128, mwin], bf16, tag="iota")
            nc.scalar.dma_start(out=iota_t[:], in_=iota_d[:])
            zeros_t = cpool.tile([128, 512], f8, tag="zeros")
            nc.vector.memset(zeros_t[:], 0.0)
            ones_t = cpool.tile([128, 1], bf16, tag="ones")
            nc.vector.memset(ones_t[:], 1.0)
            seg_t = cpool.tile([128, G], bf16, tag="segoff")
            nc.scalar.dma_start(out=seg_t[:], in_=seg_d[:])
            aw_t = cpool.tile([128, G], bf16, tag="aw")
            nc.scalar.dma_start(out=aw_t[:], in_=aw_d[:])
            # epilogue constants prefetched up front so the tail is pure
            # vector work
            qn_t = cpool.tile([128, SEGS_PER_CORE], f32, tag="qnT")
            nc.scalar.dma_start(out=qn_t[:], in_=qn_d[:])
            bui_t = cpool.tile([128, SEGS_PER_CORE // 128], f32, tag="bui")
            nc.scalar.dma_start(out=bui_t[:], in_=bui_d[:])

            # warmup: a tiny gather issued first primes the Q7 SWDGE path; the
            # ~20us one-time init is engine-serial either way, so keep the
            # warmup's own gen cost minimal (128 idxs)
            widx_t = cpool.tile([128, 3584 // 16], i16, tag="widx")
            nc.sync.dma_start(out=widx_t[:], in_=widx_d[:])
            wout_t = cpool.tile([128, 1, 256], f8, tag="wout")
            nc.gpsimd.dma_gather(
                out_ap=wout_t[:],
                in_ap=xy_d[0:BUCKET_ROWS, :],
                idxs_ap=widx_t[:, 0:8],
                num_idxs=128,
                num_idxs_reg=128,
                elem_size=256,
                single_packet=False,
                queue_num=0,
            )
            psumZ = ppool.tile([128, 512], f32, tag="psumZ")
            psumR = ppool.tile([128, 512], f32, tag="psumR")
            nc.tensor.matmul(
                out=psumZ[:, 0:512], lhsT=zeros_t[:, 0:128],
                rhs=zeros_t[:, 0:512], start=True, stop=False,
            )

            for ci, (start, n, b) in enumerate(chunks):
                if stage < 1:
                    break
                nG = n // 128
                bidx = ipool.tile([128, n // 16], i16, tag="idx")
                nc.sync.dma_start(
                    out=bidx[:], in_=idx_d[:, start // 16:(start + n) // 16])
                xyt = xypool.tile([128, nG, 256], f8, tag="xyt")
                nc.gpsimd.dma_gather(
                    out_ap=xyt[:],
                    in_ap=xy_d[b * BUCKET_ROWS:(b + 1) * BUCKET_ROWS, :],
                    idxs_ap=bidx[:],
                    num_idxs=n,
                    num_idxs_reg=n,
                    elem_size=256,
                    single_packet=False,
                    queue_num=(ci + 1) % 4,
                )
                if stage < 2:
                    continue
                c0g = start // 128
                so_t = seg_t[:, c0g:c0g + nG]
                a_t = aw_t[:, c0g:c0g + nG]
                S_t = spool.tile([128, nG, mwin], f8, tag="S")
                Sp_t = spool.tile([128, nG, mwin], f8, tag="Sp")
                nc.vector.tensor_tensor(
                    out=S_t[:],
                    in0=iota_t[:].unsqueeze(1).to_broadcast((128, nG, mwin)),
                    in1=so_t[:].to_broadcast((128, nG, mwin)),
                    op=mybir.AluOpType.is_equal,
                )
                nc.vector.tensor_tensor(
                    out=Sp_t[:], in0=S_t[:],
                    in1=a_t[:].to_broadcast((128, nG, mwin)),
                    op=mybir.AluOpType.mult,
                )

                if stage < 3:
                    continue
                for u in range(nG):
                    g = start // 128 + u
                    if not any_real[g]:
                        continue
                    Ag = int(A[g])
                    w = min(mwin, SEGS_PER_CORE - Ag)
                    nc.tensor.matmul(
                        out=psumZ[:, Ag:Ag + w],
                        lhsT=xyt[:, u, 0:128],
                        rhs=Sp_t[:, u, 0:w],
                        start=False, stop=False,
                    )
                    nc.tensor.matmul(
                        out=psumZ[:, Ag:Ag + w],
                        lhsT=xyt[:, u, 128:256],
                        rhs=S_t[:, u, 0:w],
                        start=False, stop=False,
                    )

            # close accumulation group (full-width, required before reads)
            nc.tensor.matmul(
                out=psumZ[:, 0:512], lhsT=zeros_t[:, 0:128],
                rhs=zeros_t[:, 0:512], start=False, stop=True,
            )

            # epilogue: rui = bui + sum_f qnT * psumZ, reduced to partition-
            # major [128, 4] via per-bank ones matmuls
            n_banks = SEGS_PER_CORE // 128
            red_t = epool.tile([128, n_banks], f32, tag="red")
            if stage >= 4:
                m_t = epool.tile([128, SEGS_PER_CORE], bf16, tag="m")
                nc.vector.tensor_tensor(
                    out=m_t[:], in0=psumZ[:, 0:512], in1=qn_t[:],
                    op=mybir.AluOpType.mult,
                )
                for k in range(n_banks):
                    nc.tensor.matmul(
                        out=psumR[0:128, k:k + 1],
                        lhsT=m_t[:, 128 * k:128 * (k + 1)],
                        rhs=ones_t[:, 0:1],
                        start=True, stop=True,
                    )
                nc.vector.tensor_add(red_t[:], psumR[0:128, 0:n_banks],
                                     bui_t[:])
            else:
                nc.vector.tensor_copy(out=red_t[:], in_=bui_t[:])
            for k in range(n_banks):
                nc.sync.dma_start(out=out_d[128 * k:128 * (k + 1)],
                                  in_=red_t[:, k:k + 1])

    nc.compile()
    return nc


def kernel(bu, bi, Q, X, Y, user, item, imp_items, imp_ratings, segment_ids,
           _sim=False, _stage=4):
    bu = np.asarray(bu, np.float32)
    bi = np.asarray(bi, np.float32)
    Q = np.asarray(Q, np.float32)
    X = np.asarray(X, np.float32)
    Y = np.asarray(Y, np.float32)
    user = np.asarray(user).astype(np.int64)
    item = np.asarray(item).astype(np.int64)
    imp_items = np.asarray(imp_items).astype(np.int64)
    imp_ratings = np.asarray(imp_ratings).astype(np.int64)
    segment_ids = np.asarray(segment_ids).astype(np.int64)

    in_maps, meta = _host_prep(bu, bi, Q, X, Y, user, item, imp_items,
                               imp_ratings, segment_ids)
    nc = _build_graph(meta, stage=_stage)

    if _sim:
        from concourse import bass_interp
        sim = bass_interp.CoreSim(nc)
        sim.assign_tensors(in_maps[0])
        sim.simulate()
        out0 = np.array(sim.tensor("out"))
        return sim, out0, in_maps, meta

    from concourse.bass_utils import run_bass_kernel_spmd
    res = run_bass_kernel_spmd(nc, in_maps, core_ids=list(range(N_CORES)),
                               trace=False)
    out = np.concatenate([res.results[m]["out"] for m in range(N_CORES)])
    return out.astype(np.float32)
